# revision 3
# baseline (speedup 1.0000x reference)
"""Trainium2 Bass kernel for nn_Network_23922967839459 (gnn_message_passing).

Structure exploited: the l1 ensemble state never depends on l2 or W, so the
20 per-step spike matvecs collapse into ONE batched matmul
    Y[T, N2] = SP[T, N1] @ W[N1, N2]
after simulating all T steps of l1 on-chip.  W is column-sharded over the 8
cores (tensor parallel over N2, zero collectives); each core redundantly
simulates l1 (cheap), does the matmul for its 512 columns, then runs the l2
recurrence for its own slice.

W is cast to bf16 on the host: spikes are {0,1} so every product is exact,
accumulation is fp32 in PSUM, and the only error is bf16 rounding of W
itself (~1e-5 on y, measured 500x below the smallest l2 threshold margin).
This halves the HBM traffic, which is the roofline for this kernel.

Layouts (per core):
  l1 neuron n  <->  (partition p = n % 128, column c = n // 128); the host
  pre-permutes x so DMAs are contiguous, and K-tile k of the matmul is
  exactly column k of the l1 layout.
  l2 neuron j  <->  (partition p = j % 128, block jb = j // 128) after a
  PE transpose of the matmul output.
"""

import os
import sys

sys.path.insert(0, "/opt/trn_rl_repo")

import numpy as np
import ml_dtypes

import concourse.bass as bass
import concourse.mybir as mybir
from concourse import bacc
from concourse.tile import TileContext
from concourse.bass_utils import run_bass_kernel_spmd
from concourse.masks import make_identity

F32 = mybir.dt.float32
BF16 = mybir.dt.bfloat16
ALU = mybir.AluOpType
ACTF = mybir.ActivationFunctionType

# fl(1/1.05) - 1 computed so that 1.0f + C_FAC == fl(1/1.05) exactly
# (Sterbenz: fl(1/1.05) in [0.5, 1) makes the subtraction exact).
C_INV105 = np.float32(1.0) / np.float32(1.05)
C_FAC = float(C_INV105 - np.float32(1.0))


class Cfg:
    def __init__(self, T=20, COLS=512, N2L=512, FC=256, WT=8, W_BUFS=8, REPS=1):
        self.T = T            # time steps
        self.COLS = COLS      # l1 columns = K tiles; N1 = 128*COLS
        self.N2L = N2L        # local l2 width (columns of W slice)
        self.FC = FC          # phase-A chunk width (columns)
        self.WT = WT          # K-tiles per W DMA
        self.W_BUFS = W_BUFS  # W streaming pool depth
        self.REPS = REPS      # repeat whole computation (timing only)
        assert COLS % FC == 0 and COLS % WT == 0
        assert N2L % 128 == 0
        self.N1 = 128 * COLS
        self.JB = N2L // 128  # l2 partition blocks


def _emit_ens_step(nc, sl, x_ap, st, tmp, sp_out_ap):
    """One ensemble step on [128, F] slices.

    st: dict of state APs (act, u, thr, freq) already sliced to [128, F].
    tmp: dict of temp APs sliced to [128, F].
    x_ap: input slice [128, F].
    sp_out_ap: where to write the spike (bf16 or f32), or None.

    u is the gain state shifted by +0.3 (so the refractory reset is a pure
    multiply by nsp): u = gain + 0.3.
    """
    act, u, thr, freq = st["act"], st["u"], st["thr"], st["freq"]
    m, act2, nsp, gt, fac, g05 = (
        tmp["m"], tmp["act2"], tmp["nsp"], tmp["gt"], tmp["fac"], tmp["g05"],
    )
    # gain' = 0.8*gain + 0.2  ->  u' = 0.8*u + 0.26   (ACT)
    nc.scalar.activation(u, u, ACTF.Copy, bias=0.26, scale=0.8)
    # m = (u' - 0.3) * x = gain' * x                  (DVE fused)
    nc.vector.scalar_tensor_tensor(m, u, -0.3, x_ap, ALU.add, ALU.mult)
    # act = 0.9*act + m                               (DVE fused)
    nc.vector.scalar_tensor_tensor(act, act, 0.9, m, ALU.mult, ALU.add)
    # act2 = act + 0.05   (pre-reset activation)      (ACT)
    nc.scalar.activation(act2, act, ACTF.Copy, bias=0.05, scale=1.0)
    # nsp = act2 <= thr   (1.0 if NO spike)           (DVE)
    nc.vector.tensor_tensor(nsp, act2, thr, ALU.is_le)
    if sp_out_ap is not None:
        # spike = 1 - nsp                             (DVE ts)
        nc.vector.tensor_scalar(sp_out_ap, nsp, -1.0, 1.0, ALU.mult, ALU.add)
    # g05 = 0.05*spike = -0.05*nsp + 0.05             (ACT)
    nc.scalar.activation(g05, nsp, ACTF.Copy, bias=0.05, scale=-0.05)
    # freq = 0.95*freq + g05                          (DVE fused)
    nc.vector.scalar_tensor_tensor(freq, freq, 0.95, g05, ALU.mult, ALU.add)
    # gt = freq > 0.1                                 (DVE ts)
    nc.vector.tensor_scalar(gt, freq, 0.1, None, ALU.is_gt)
    # freq is never exactly 0.1 (measured margin 2.6e-5), so lt = 1-gt and
    # fac = 1 + C_FAC*(1-gt) = (1+C_FAC) - C_FAC*gt   (ACT)
    nc.scalar.activation(fac, gt, ACTF.Copy, bias=1.0 + C_FAC, scale=-C_FAC)
    # thr = (thr + 0.05*gt) * fac                     (DVE fused + DVE)
    nc.vector.scalar_tensor_tensor(thr, gt, 0.05, thr, ALU.mult, ALU.add)
    nc.vector.tensor_tensor(thr, thr, fac, ALU.mult)
    # act = act2*nsp ; u = u'*nsp  (zero reset / refractory)
    nc.vector.tensor_tensor(act, act2, nsp, ALU.mult)
    nc.vector.tensor_tensor(u, u, nsp, ALU.mult)


def build_nc(cfg: Cfg):
    nc = bacc.Bacc("TRN2", target_bir_lowering=False, debug=False, num_devices=8)
    T, COLS, N2L, FC, WT = cfg.T, cfg.COLS, cfg.N2L, cfg.FC, cfg.WT
    KT = COLS  # K tiles

    # x pre-permuted on host: x_dram[t, p, c] = x_orig[t, c*128 + p]
    x_dram = nc.dram_tensor("x", [T, 128, COLS], F32, kind="ExternalInput").ap()
    w_dram = nc.dram_tensor("w", [cfg.N1, N2L], BF16, kind="ExternalInput").ap()
    o_dram = nc.dram_tensor("o", [128, cfg.JB, T], F32, kind="ExternalOutput").ap()

    x_perm = x_dram.rearrange("t p c -> p t c")
    w_view = w_dram.rearrange("(i j p) n -> i p j n", p=128, j=WT)

    with TileContext(nc) as tc:
        with (
            tc.tile_pool(name="persist", bufs=1) as pp,
            tc.tile_pool(name="wpool", bufs=cfg.W_BUFS) as wp,
            tc.tile_pool(name="psum", bufs=1, space="PSUM") as psp,
            tc.tile_pool(name="psum_t", bufs=2, space="PSUM") as pst,
        ):
            ident = pp.tile([128, 128], F32)
            make_identity(nc, ident[:])

            x_sb = pp.tile([128, T, COLS], F32)
            sp3 = pp.tile([128, KT, T], BF16)

            st = {k: pp.tile([128, COLS], F32, name=f"st_{k}")
                  for k in ("act", "u", "thr", "freq")}
            tmp = {k: pp.tile([128, COLS], F32, name=f"tmp_{k}")
                   for k in ("m", "act2", "nsp", "gt", "fac", "g05")}

            # l2 state [128, JB]
            st2 = {k: pp.tile([128, cfg.JB], F32, name=f"st2_{k}")
                   for k in ("u", "thr", "freq")}
            tmp2 = {k: pp.tile([128, cfg.JB], F32, name=f"tmp2_{k}")
                    for k in ("m", "act2", "nsp", "gt", "fac", "g05")}
            act2_init = pp.tile([128, cfg.JB], F32)

            y_sb = pp.tile([T, N2L], F32)
            yt_sb = pp.tile([128, cfg.JB, T], F32)
            hist = pp.tile([128, cfg.JB, T], F32)

            for rep in range(cfg.REPS):
                # ---- init state ----
                nc.gpsimd.memset(st["act"][:], 0.0)
                nc.gpsimd.memset(st["u"][:], 1.3)
                nc.gpsimd.memset(st["thr"][:], 1.0)
                nc.gpsimd.memset(st["freq"][:], 0.0)
                nc.gpsimd.memset(act2_init[:], 0.0)
                nc.gpsimd.memset(st2["u"][:], 1.3)
                nc.gpsimd.memset(st2["thr"][:], 1.0)
                nc.gpsimd.memset(st2["freq"][:], 0.0)

                # ---- x loads (per phase-A chunk) ----
                for c0 in range(0, COLS, FC):
                    nc.sync.dma_start(
                        x_sb[:, :, c0:c0 + FC], x_perm[:, :, c0:c0 + FC]
                    )

                # ---- phase A: l1 dynamics, chunked over columns ----
                for c0 in range(0, COLS, FC):
                    sl = slice(c0, c0 + FC)
                    stc = {k: v[:, sl] for k, v in st.items()}
                    tmpc = {k: v[:, sl] for k, v in tmp.items()}
                    for t in range(T):
                        _emit_ens_step(
                            nc, sl, x_sb[:, t, sl], stc, tmpc,
                            sp3[:, sl, t],
                        )

                # ---- phase B: Y[T, N2L] = sum_k SP_k^T @ W_k ----
                psum_y = psp.tile([T, N2L], F32)
                for i in range(KT // WT):
                    wt = wp.tile([128, WT, N2L], BF16)
                    nc.sync.dma_start(wt[:], w_view[i])
                    for j in range(WT):
                        k = i * WT + j
                        nc.tensor.matmul(
                            psum_y[:],
                            sp3[:, k, :],
                            wt[:, j, :],
                            start=(k == 0),
                            stop=(k == KT - 1),
                        )

                nc.vector.tensor_copy(y_sb[:], psum_y[:])

                # ---- transpose Y -> [128, JB, T] ----
                for jb in range(cfg.JB):
                    ps = pst.tile([128, T], F32)
                    nc.tensor.transpose(
                        ps[:], y_sb[:, jb * 128:(jb + 1) * 128], ident[:T, :T]
                    )
                    nc.vector.tensor_copy(yt_sb[:, jb, :], ps[:])

                # ---- phase C: l2 dynamics; act state lives in hist ----
                for t in range(T):
                    act_prev = act2_init[:] if t == 0 else hist[:, :, t - 1]
                    stc2 = dict(st2)
                    stc2["act"] = hist[:, :, t]
                    # act = 0.9*act_prev + m needs separate in/out: do the
                    # fused ops with explicit in/out ordering
                    u2, thr2, freq2 = st2["u"], st2["thr"], st2["freq"]
                    m2, act2_2, nsp2, gt2, fac2, g05_2 = (
                        tmp2["m"], tmp2["act2"], tmp2["nsp"], tmp2["gt"],
                        tmp2["fac"], tmp2["g05"],
                    )
                    acth = hist[:, :, t]
                    y_t = yt_sb[:, :, t]
                    nc.scalar.activation(u2, u2, ACTF.Copy, bias=0.26, scale=0.8)
                    nc.vector.scalar_tensor_tensor(m2, u2, -0.3, y_t, ALU.add, ALU.mult)
                    nc.vector.scalar_tensor_tensor(acth, act_prev, 0.9, m2, ALU.mult, ALU.add)
                    nc.scalar.activation(act2_2, acth, ACTF.Copy, bias=0.05, scale=1.0)
                    nc.vector.tensor_tensor(nsp2, act2_2, thr2, ALU.is_le)
                    nc.scalar.activation(g05_2, nsp2, ACTF.Copy, bias=0.05, scale=-0.05)
                    nc.vector.scalar_tensor_tensor(freq2, freq2, 0.95, g05_2, ALU.mult, ALU.add)
                    nc.vector.tensor_scalar(gt2, freq2, 0.1, None, ALU.is_gt)
                    nc.scalar.activation(fac2, gt2, ACTF.Copy, bias=1.0 + C_FAC, scale=-C_FAC)
                    nc.vector.scalar_tensor_tensor(thr2, gt2, 0.05, thr2, ALU.mult, ALU.add)
                    nc.vector.tensor_tensor(thr2, thr2, fac2, ALU.mult)
                    nc.vector.tensor_tensor(acth, act2_2, nsp2, ALU.mult)
                    nc.vector.tensor_tensor(u2, u2, nsp2, ALU.mult)

                nc.sync.dma_start(o_dram[:], hist[:])

    nc.compile()
    return nc


# ---------------------------------------------------------------------------
# Host side
# ---------------------------------------------------------------------------

T, N1, N2, NCORES = 20, 65536, 4096, 8
N2L = N2 // NCORES

_cached_nc = None


def _get_nc():
    global _cached_nc
    if _cached_nc is None:
        _cached_nc = build_nc(Cfg())
    return _cached_nc


def prep_inputs(x_seq: np.ndarray, W: np.ndarray):
    """Host-side shard prep: permute x, cast W to bf16, slice columns."""
    assert x_seq.shape == (T, N1) and W.shape == (N1, N2)
    COLS = N1 // 128
    # x_perm[t, p, c] = x_seq[t, c*128 + p]
    x_perm = np.ascontiguousarray(
        x_seq.reshape(T, COLS, 128).transpose(0, 2, 1)
    ).astype(np.float32)
    Wb = W.astype(ml_dtypes.bfloat16)
    in_maps = []
    for c in range(NCORES):
        in_maps.append({
            "x": x_perm,
            "w": np.ascontiguousarray(Wb[:, c * N2L:(c + 1) * N2L]),
        })
    return in_maps


def assemble_output(results):
    """results: list of per-core out maps with 'o' [128, JB, T]."""
    out = np.empty((T, N2), dtype=np.float32)
    for c in range(NCORES):
        o = results[c]["o"]  # [128, JB, T], j = jb*128 + p
        out[:, c * N2L:(c + 1) * N2L] = o.transpose(2, 1, 0).reshape(T, N2L)
    return out


def kernel(x_seq: np.ndarray, W: np.ndarray) -> np.ndarray:
    nc = _get_nc()
    in_maps = prep_inputs(np.asarray(x_seq), np.asarray(W))
    res = run_bass_kernel_spmd(nc, in_maps, core_ids=list(range(NCORES)))
    return assemble_output(res.results)


# revision 8
# speedup vs baseline: 11.9140x; 11.9140x over previous
"""Trainium2 Bass kernel for nn_Network_23922967839459 (gnn_message_passing).

Structure exploited: the l1 ensemble state never depends on l2 or W, so the
20 per-step spike matvecs collapse into ONE batched matmul
    Y[T, N2] = SP[T, N1] @ W[N1, N2]
after simulating all T steps of l1 on-chip.  W is column-sharded over the 8
cores (tensor parallel over N2, zero collectives); each core redundantly
simulates l1 (cheap), does the matmul for its 512 columns, then runs the l2
recurrence for its own slice.

W is cast to bf16 on the host: spikes are {0,1} so every product is exact,
accumulation is fp32 in PSUM, and the only error is bf16 rounding of W
itself (~1e-5 on y, measured 500x below the smallest l2 threshold margin).
This halves the HBM traffic, which is the roofline for this kernel.

Layouts (per core):
  l1 neuron n  <->  (partition p = n % 128, column c = n // 128); the host
  pre-permutes x so DMAs are contiguous, and K-tile k of the matmul is
  exactly column k of the l1 layout.
  l2 neuron j  <->  (partition p = j % 128, block jb = j // 128) after a
  PE transpose of the matmul output.
"""

import os
import sys

sys.path.insert(0, "/opt/trn_rl_repo")

import numpy as np
import ml_dtypes

import concourse.bass as bass
import concourse.mybir as mybir
from concourse import bacc
from concourse.tile import TileContext
from concourse.bass_utils import run_bass_kernel_spmd
from concourse.masks import make_identity

F32 = mybir.dt.float32
BF16 = mybir.dt.bfloat16
ALU = mybir.AluOpType
ACTF = mybir.ActivationFunctionType

# fl(1/1.05) - 1 computed so that 1.0f + C_FAC == fl(1/1.05) exactly
# (Sterbenz: fl(1/1.05) in [0.5, 1) makes the subtraction exact).
C_INV105 = np.float32(1.0) / np.float32(1.05)
C_FAC = float(C_INV105 - np.float32(1.0))


class Cfg:
    def __init__(self, T=20, COLS=512, N2L=512, FC=256, WT=8, W_BUFS=12, REPS=1,
                 DO_A=True, DO_B=True, DO_C=True, DMA_ONLY=False):
        self.T = T            # time steps
        self.COLS = COLS      # l1 columns = K tiles; N1 = 128*COLS
        self.N2L = N2L        # local l2 width (columns of W slice)
        self.FC = FC          # phase-A chunk width (columns)
        self.WT = WT          # K-tiles per W DMA
        self.W_BUFS = W_BUFS  # W streaming pool depth
        self.REPS = REPS      # repeat whole computation (timing only)
        self.DO_A, self.DO_B, self.DO_C, self.DMA_ONLY = DO_A, DO_B, DO_C, DMA_ONLY
        assert COLS % FC == 0 and COLS % WT == 0
        assert N2L % 128 == 0
        self.N1 = 128 * COLS
        self.JB = N2L // 128  # l2 partition blocks


def _emit_ens_step(nc, sl, x_ap, st, tmp, sp_out_ap):
    """One ensemble step on [128, F] slices.

    st: dict of state APs (act, u, thr, freq) already sliced to [128, F].
    tmp: dict of temp APs sliced to [128, F].
    x_ap: input slice [128, F].
    sp_out_ap: where to write the spike (bf16 or f32), or None.

    u is the gain state shifted by +0.3 (so the refractory reset is a pure
    multiply by nsp): u = gain + 0.3.
    """
    act, u, thr, freq = st["act"], st["u"], st["thr"], st["freq"]
    m, act2, nsp, gt, fac, g05 = (
        tmp["m"], tmp["act2"], tmp["nsp"], tmp["gt"], tmp["fac"], tmp["g05"],
    )
    # gain' = 0.8*gain + 0.2  ->  u' = 0.8*u + 0.26   (ACT)
    nc.scalar.activation(u, u, ACTF.Copy, bias=0.26, scale=0.8)
    # m = (u' - 0.3) * x = gain' * x                  (DVE fused)
    nc.vector.scalar_tensor_tensor(m, u, -0.3, x_ap, ALU.add, ALU.mult)
    # act = 0.9*act + m                               (DVE fused)
    nc.vector.scalar_tensor_tensor(act, act, 0.9, m, ALU.mult, ALU.add)
    # act2 = act + 0.05   (pre-reset activation)      (ACT)
    nc.scalar.activation(act2, act, ACTF.Copy, bias=0.05, scale=1.0)
    # nsp = act2 <= thr   (1.0 if NO spike)           (DVE)
    nc.vector.tensor_tensor(nsp, act2, thr, ALU.is_le)
    if sp_out_ap is not None:
        # spike = 1 - nsp                             (DVE ts)
        nc.vector.tensor_scalar(sp_out_ap, nsp, -1.0, 1.0, ALU.mult, ALU.add)
    # g05 = 0.05*spike = -0.05*nsp + 0.05             (ACT)
    nc.scalar.activation(g05, nsp, ACTF.Copy, bias=0.05, scale=-0.05)
    # freq = 0.95*freq + g05                          (DVE fused)
    nc.vector.scalar_tensor_tensor(freq, freq, 0.95, g05, ALU.mult, ALU.add)
    # gt = freq > 0.1                                 (DVE ts)
    nc.vector.tensor_scalar(gt, freq, 0.1, None, ALU.is_gt)
    # freq is never exactly 0.1 (measured margin 2.6e-5), so lt = 1-gt and
    # fac = 1 + C_FAC*(1-gt) = (1+C_FAC) - C_FAC*gt   (ACT)
    nc.scalar.activation(fac, gt, ACTF.Copy, bias=1.0 + C_FAC, scale=-C_FAC)
    # thr = (thr + 0.05*gt) * fac                     (DVE fused + DVE)
    nc.vector.scalar_tensor_tensor(thr, gt, 0.05, thr, ALU.mult, ALU.add)
    nc.vector.tensor_tensor(thr, thr, fac, ALU.mult)
    # act = act2*nsp ; u = u'*nsp  (zero reset / refractory)
    nc.vector.tensor_tensor(act, act2, nsp, ALU.mult)
    nc.vector.tensor_tensor(u, u, nsp, ALU.mult)


def build_nc(cfg: Cfg):
    nc = bacc.Bacc("TRN2", target_bir_lowering=False, debug=False, num_devices=8)
    T, COLS, N2L, FC, WT = cfg.T, cfg.COLS, cfg.N2L, cfg.FC, cfg.WT
    KT = COLS  # K tiles

    # x pre-permuted on host: x_dram[p, c, t] = x_orig[t, c*128 + p]
    # (chunk slices are then fully contiguous per partition)
    x_dram = nc.dram_tensor("x", [128, COLS, T], F32, kind="ExternalInput").ap()
    # W pre-permuted on host to tile-major: w_dram[i, p, j*N2L + n] =
    # W[(i*WT + j)*128 + p, n]  -> every DMA is one contiguous block
    w_dram = nc.dram_tensor("w", [KT // WT, 128, WT * N2L], BF16,
                            kind="ExternalInput").ap()
    o_dram = nc.dram_tensor("o", [128, cfg.JB, T], F32, kind="ExternalOutput").ap()

    with TileContext(nc) as tc:
        with (
            tc.tile_pool(name="persist", bufs=1) as pp,
            tc.tile_pool(name="wpool", bufs=cfg.W_BUFS) as wp,
            tc.tile_pool(name="psum", bufs=1, space="PSUM") as psp,
            tc.tile_pool(name="psum_t", bufs=2, space="PSUM") as pst,
        ):
            ident = pp.tile([128, 128], F32)
            make_identity(nc, ident[:])

            x_sb = pp.tile([128, COLS, T], F32)
            sp3 = pp.tile([128, KT, T], BF16)

            st = {k: pp.tile([128, COLS], F32, name=f"st_{k}")
                  for k in ("act", "u", "thr", "freq")}
            tmp = {k: pp.tile([128, COLS], F32, name=f"tmp_{k}")
                   for k in ("m", "act2", "nsp", "gt", "fac", "g05")}

            # l2 state [128, JB]
            st2 = {k: pp.tile([128, cfg.JB], F32, name=f"st2_{k}")
                   for k in ("u", "thr", "freq")}
            tmp2 = {k: pp.tile([128, cfg.JB], F32, name=f"tmp2_{k}")
                    for k in ("m", "act2", "nsp", "gt", "fac", "g05")}
            act2_init = pp.tile([128, cfg.JB], F32)

            y_sb = pp.tile([T, N2L], F32)
            yt_sb = pp.tile([128, cfg.JB, T], F32)
            hist = pp.tile([128, cfg.JB, T], F32)

            for rep in range(cfg.REPS):
                # ---- init state ----
                nc.gpsimd.memset(st["act"][:], 0.0)
                nc.gpsimd.memset(st["u"][:], 1.3)
                nc.gpsimd.memset(st["thr"][:], 1.0)
                nc.gpsimd.memset(st["freq"][:], 0.0)
                nc.gpsimd.memset(act2_init[:], 0.0)
                nc.gpsimd.memset(st2["u"][:], 1.3)
                nc.gpsimd.memset(st2["thr"][:], 1.0)
                nc.gpsimd.memset(st2["freq"][:], 0.0)

                # ---- x loads (per phase-A chunk, contiguous) ----
                for c0 in range(0, COLS, FC):
                    nc.sync.dma_start(
                        x_sb[:, c0:c0 + FC, :], x_dram[:, c0:c0 + FC, :]
                    )

                # ---- phase A: l1 dynamics, chunked over columns ----
                if cfg.DO_A:
                    for c0 in range(0, COLS, FC):
                        sl = slice(c0, c0 + FC)
                        stc = {k: v[:, sl] for k, v in st.items()}
                        tmpc = {k: v[:, sl] for k, v in tmp.items()}
                        for t in range(T):
                            _emit_ens_step(
                                nc, sl, x_sb[:, sl, t], stc, tmpc,
                                sp3[:, sl, t],
                            )
                else:
                    nc.gpsimd.memset(sp3[:], 0.0)

                # ---- phase B: Y[T, N2L] = sum_k SP_k^T @ W_k ----
                psum_y = psp.tile([T, N2L], F32)
                if cfg.DO_B:
                    for i in range(KT // WT):
                        wt = wp.tile([128, WT * N2L], BF16)
                        nc.sync.dma_start(wt[:], w_dram[i])
                        if cfg.DMA_ONLY:
                            nc.vector.tensor_copy(y_sb[:1, :1], wt[:1, :1])
                            continue
                        for j in range(WT):
                            k = i * WT + j
                            nc.tensor.matmul(
                                psum_y[:],
                                sp3[:, k, :],
                                wt[:, j * N2L:(j + 1) * N2L],
                                start=(k == 0),
                                stop=(k == KT - 1),
                            )
                if not cfg.DO_B or cfg.DMA_ONLY:
                    nc.vector.memset(psum_y[:], 0.0)
                nc.vector.tensor_copy(y_sb[:], psum_y[:])

                # ---- transpose Y -> [128, JB, T] ----
                for jb in range(cfg.JB):
                    ps = pst.tile([128, T], F32)
                    nc.tensor.transpose(
                        ps[:], y_sb[:, jb * 128:(jb + 1) * 128], ident[:T, :T]
                    )
                    nc.vector.tensor_copy(yt_sb[:, jb, :], ps[:])

                # ---- phase C: l2 dynamics; act state lives in hist ----
                if not cfg.DO_C:
                    nc.gpsimd.memset(hist[:], 0.0)
                for t in range(T if cfg.DO_C else 0):
                    act_prev = act2_init[:] if t == 0 else hist[:, :, t - 1]
                    stc2 = dict(st2)
                    stc2["act"] = hist[:, :, t]
                    # act = 0.9*act_prev + m needs separate in/out: do the
                    # fused ops with explicit in/out ordering
                    u2, thr2, freq2 = st2["u"], st2["thr"], st2["freq"]
                    m2, act2_2, nsp2, gt2, fac2, g05_2 = (
                        tmp2["m"], tmp2["act2"], tmp2["nsp"], tmp2["gt"],
                        tmp2["fac"], tmp2["g05"],
                    )
                    acth = hist[:, :, t]
                    y_t = yt_sb[:, :, t]
                    nc.scalar.activation(u2, u2, ACTF.Copy, bias=0.26, scale=0.8)
                    nc.vector.scalar_tensor_tensor(m2, u2, -0.3, y_t, ALU.add, ALU.mult)
                    nc.vector.scalar_tensor_tensor(acth, act_prev, 0.9, m2, ALU.mult, ALU.add)
                    nc.scalar.activation(act2_2, acth, ACTF.Copy, bias=0.05, scale=1.0)
                    nc.vector.tensor_tensor(nsp2, act2_2, thr2, ALU.is_le)
                    nc.scalar.activation(g05_2, nsp2, ACTF.Copy, bias=0.05, scale=-0.05)
                    nc.vector.scalar_tensor_tensor(freq2, freq2, 0.95, g05_2, ALU.mult, ALU.add)
                    nc.vector.tensor_scalar(gt2, freq2, 0.1, None, ALU.is_gt)
                    nc.scalar.activation(fac2, gt2, ACTF.Copy, bias=1.0 + C_FAC, scale=-C_FAC)
                    nc.vector.scalar_tensor_tensor(thr2, gt2, 0.05, thr2, ALU.mult, ALU.add)
                    nc.vector.tensor_tensor(thr2, thr2, fac2, ALU.mult)
                    nc.vector.tensor_tensor(acth, act2_2, nsp2, ALU.mult)
                    nc.vector.tensor_tensor(u2, u2, nsp2, ALU.mult)

                nc.sync.dma_start(o_dram[:], hist[:])

    nc.compile()
    return nc


# ---------------------------------------------------------------------------
# Host side
# ---------------------------------------------------------------------------

T, N1, N2, NCORES = 20, 65536, 4096, 8
N2L = N2 // NCORES

_cached_nc = None


def _get_nc():
    global _cached_nc
    if _cached_nc is None:
        _cached_nc = build_nc(Cfg())
    return _cached_nc


def prep_inputs(x_seq: np.ndarray, W: np.ndarray, cfg=None):
    """Host-side shard prep: permute x, cast W to bf16 tile-major slices."""
    if cfg is None:
        cfg = Cfg()
    assert x_seq.shape == (T, N1) and W.shape == (N1, N2)
    COLS = N1 // 128
    WT = cfg.WT
    # x_perm[p, c, t] = x_seq[t, c*128 + p]
    x_perm = np.ascontiguousarray(
        x_seq.reshape(T, COLS, 128).transpose(2, 1, 0)
    ).astype(np.float32)
    Wb = W.astype(ml_dtypes.bfloat16)
    in_maps = []
    for c in range(NCORES):
        ws = Wb[:, c * N2L:(c + 1) * N2L]  # [N1, N2L]
        wt = np.ascontiguousarray(
            ws.reshape(COLS // WT, WT, 128, N2L).transpose(0, 2, 1, 3)
        ).reshape(COLS // WT, 128, WT * N2L)
        in_maps.append({"x": x_perm, "w": wt})
    return in_maps


def assemble_output(results):
    """results: list of per-core out maps with 'o' [128, JB, T]."""
    out = np.empty((T, N2), dtype=np.float32)
    for c in range(NCORES):
        o = results[c]["o"]  # [128, JB, T], j = jb*128 + p
        out[:, c * N2L:(c + 1) * N2L] = o.transpose(2, 1, 0).reshape(T, N2L)
    return out


def kernel(x_seq: np.ndarray, W: np.ndarray) -> np.ndarray:
    nc = _get_nc()
    in_maps = prep_inputs(np.asarray(x_seq), np.asarray(W))
    res = run_bass_kernel_spmd(nc, in_maps, core_ids=list(range(NCORES)))
    return assemble_output(res.results)
